# revision 1
# baseline (speedup 1.0000x reference)
"""Trainium2 Bass kernel for nn_KCLWONegLoss.

Reference math (all f32):
    sums    = embs.sum(axis=1)                          # [64, 512]
    pos[p]  = cos(sums[p], sums[p+8])                   # p in 0..55
    a       = g1[neg1]; b = g2[neg2]                    # [56, 32, 512]
    sim[p,d]= cos over K axis (32) of a[p,:,d], b[p,:,d]
    num     = exp(pos/0.1)
    den     = num + sum_d exp(sim/0.1)
    loss    = 2 * sum_p (log(den) - pos/0.1)

Sharding: data-parallel over the D=64 group axis (8 groups/core) for the
embs reduction; the 56 positive pairs are sharded 7/core, with each core
receiving only its 7*32 gathered rows of g1/g2 (row-gather done host-side
at shard-build time; the device still reads every gathered byte from HBM).
Per-core device outputs: the 8 group-sum vectors [8,512] and the 7 partial
negative-denominator sums [7]. The final 56 cosines + log-sum (≈0.1 Mflop)
are assembled on host in float64.
"""

import numpy as np

D, NG, DIM = 64, 256, 512
L, K = 8, 32
P = D - L               # 56 positive pairs
TEMP = 0.1
EPS = 1e-8
N_CORES = 8
GPC = D // N_CORES      # 8 groups per core
PPC = P // N_CORES      # 7 pairs per core
ROWS = PPC * K          # 224 gathered rows per core, padded to 256

_PROGRAM = None         # cached compiled Bass program
LAST_RESULTS = None     # BassKernelResults of the most recent run (for test.py)


def _build_program():
    import concourse.bass as bass
    import concourse.tile as tile
    from concourse.tile import add_dep_helper
    from concourse import bacc, mybir

    f32 = mybir.dt.float32
    f32r = mybir.dt.float32r
    nc = bacc.Bacc("TRN2", target_bir_lowering=False, debug=False)

    embs_t = nc.dram_tensor("embs_s", [GPC, NG, DIM], f32, kind="ExternalInput")
    gab_t = nc.dram_tensor("gab", [4, 128, DIM], f32, kind="ExternalInput")
    consts_t = nc.dram_tensor("consts", [128, 80], f32, kind="ExternalInput")
    sums_t = nc.dram_tensor("sums_out", [GPC, DIM], f32, kind="ExternalOutput")
    den_t = nc.dram_tensor("den_out", [8, 1], f32, kind="ExternalOutput")

    with tile.TileContext(nc) as tc:
        with (
            tc.tile_pool(name="pool", bufs=1) as pool,
            tc.tile_pool(name="psum", bufs=1, space=bass.MemorySpace.PSUM) as psum,
        ):
            # consts columns (see kernel() for values):
            #   8g..8g+8   : selector S_g — all-ones in column g, else 0
            #   64..72     : block-ones for pairs 0..3 (col m = rows 32m..32m+32)
            #   72..80     : block-ones for pairs 4..7 (col 4+m likewise)
            consts = pool.tile([128, 80], f32r, tag="consts")
            nc.sync.dma_start(consts[:], consts_t.ap().bitcast(f32r))
            blk = [consts[:, 64:72], consts[:, 72:80]]

            # --- negative path: all 4 gather tiles in one packed DMA ---
            gab = pool.tile([128, 4, DIM], f32, tag="gab")
            nc.sync.dma_start(gab[:], gab_t.ap().rearrange("t p d -> p t d"))
            ab = [(gab[:, 0, :], gab[:, 2, :]), (gab[:, 1, :], gab[:, 3, :])]

            # --- embs shard: one DMA per group, [128, 2, 512] (n = h*128+p).
            # All 8 chunks stream in parallel (fair-shared queues). The
            # two n-halves of each chunk are pre-reduced on the otherwise
            # idle Vector engine so only 8 selector-matmuls remain after
            # the stream ends.
            etiles = []
            for g in range(GPC):
                e = pool.tile([128, 2, DIM], f32r, tag=f"e{g}")
                nc.sync.dma_start(
                    e[:], embs_t.ap()[g].rearrange("(h p) d -> p h d", p=128).bitcast(f32r)
                )
                etiles.append(e)

            # --- negative path compute ---
            dot_ps = psum.tile([8, DIM], f32, tag="dot")
            asq_ps = psum.tile([8, DIM], f32, tag="asq")
            bsq_ps = psum.tile([8, DIM], f32, tag="bsq")
            for t, (a, b) in enumerate(ab):
                prod = pool.tile([128, DIM], f32r, tag=f"prod{t}")
                aa = pool.tile([128, DIM], f32r, tag=f"aa{t}")
                bb = pool.tile([128, DIM], f32r, tag=f"bb{t}")
                nc.vector.tensor_mul(prod[:], a, b)
                nc.vector.tensor_mul(aa[:], a, a)
                nc.vector.tensor_mul(bb[:], b, b)
                st, sp = (t == 0), (t == 1)
                nc.tensor.matmul(dot_ps[:], blk[t], prod[:], start=st, stop=sp)
                nc.tensor.matmul(asq_ps[:], blk[t], aa[:], start=st, stop=sp)
                nc.tensor.matmul(bsq_ps[:], blk[t], bb[:], start=st, stop=sp)

            # --- group sums: DVE-reduce the two halves, then one
            # selector-matmul per group accumulating into [8,512] ---
            sums_ps = psum.tile([GPC, DIM], f32, tag="sums")
            for g in range(GPC):
                c = pool.tile([128, DIM], f32r, tag=f"c{g}")
                with nc.allow_low_precision(reason="f32r is fp32-width; PE rounds"):
                    nc.vector.tensor_reduce(
                        c[:],
                        etiles[g].rearrange("p h d -> p d h"),
                        axis=mybir.AxisListType.X,
                        op=mybir.AluOpType.add,
                    )
                nc.tensor.matmul(
                    sums_ps[:],
                    consts[:, 8 * g:8 * g + 8],
                    c[:],
                    start=(g == 0),
                    stop=(g == GPC - 1),
                )

            # --- epilogue: sim = dot * rsqrt(asq) * rsqrt(bsq).
            # (gather pad rows are 1.0 so asq/bsq are never 0; the reference
            # eps guard can never bind for randn inputs)
            import concourse.mybir as mybir_
            AF = mybir_.ActivationFunctionType
            ai = pool.tile([8, DIM], f32, tag="ai")
            bi = pool.tile([8, DIM], f32, tag="bi")
            nc.scalar.activation(ai[:], asq_ps[:], AF.Abs_reciprocal_sqrt)
            nc.scalar.activation(bi[:], bsq_ps[:], AF.Abs_reciprocal_sqrt)
            tmp = pool.tile([8, DIM], f32, tag="tmp")
            nc.vector.tensor_mul(tmp[:], dot_ps[:], ai[:])
            sim = pool.tile([8, DIM], f32, tag="sim")
            nc.vector.tensor_mul(sim[:], tmp[:], bi[:])
            # e = exp(sim/TEMP), den = row-sum(e) fused via accum_out
            e = pool.tile([8, DIM], f32, tag="e")
            den = pool.tile([8, 1], f32, tag="den")
            nc.scalar.activation(
                e[:], sim[:], AF.Exp,
                scale=float(1.0 / TEMP), accum_out=den[:],
            )

            sums_sb = pool.tile([GPC, DIM], f32, tag="sums_sb")
            nc.scalar.copy(sums_sb[:], sums_ps[:])
            nc.sync.dma_start(sums_t.ap(), sums_sb[:])
            nc.sync.dma_start(den_t.ap(), den[:])

    nc.compile()
    return nc


def _get_program():
    global _PROGRAM
    if _PROGRAM is None:
        _PROGRAM = _build_program()
    return _PROGRAM


def kernel(embs, g0, g1, g2, neg1, neg2, **_unused):
    global LAST_RESULTS
    from concourse.bass_utils import run_bass_kernel_spmd

    embs = np.ascontiguousarray(np.asarray(embs, dtype=np.float32))
    g1 = np.ascontiguousarray(np.asarray(g1, dtype=np.float32))
    g2 = np.ascontiguousarray(np.asarray(g2, dtype=np.float32))
    neg1 = np.asarray(neg1).astype(np.int64)
    neg2 = np.asarray(neg2).astype(np.int64)

    consts = np.zeros((128, 80), np.float32)
    for g in range(GPC):
        consts[:, 8 * g + g] = 1.0          # selector S_g, column g
    for m in range(4):
        consts[m * 32:(m + 1) * 32, 64 + m] = 1.0
        consts[m * 32:(m + 1) * 32, 72 + 4 + m] = 1.0

    in_maps = []
    for c in range(N_CORES):
        # pad rows are 1.0: the fake 8th pair then has asq=bsq=K exactly,
        # keeping rsqrt finite (its den_out row is discarded host-side)
        gab = np.ones((4, 128, DIM), np.float32)
        idx1 = neg1[c * PPC:(c + 1) * PPC].reshape(-1)
        idx2 = neg2[c * PPC:(c + 1) * PPC].reshape(-1)
        gab[:2].reshape(256, DIM)[:ROWS] = g1[idx1]
        gab[2:].reshape(256, DIM)[:ROWS] = g2[idx2]
        in_maps.append({
            "embs_s": embs[c * GPC:(c + 1) * GPC],
            "gab": gab,
            "consts": consts,
        })

    nc = _get_program()
    res = run_bass_kernel_spmd(nc, in_maps, core_ids=list(range(N_CORES)))
    LAST_RESULTS = res

    sums = np.concatenate(
        [res.results[c]["sums_out"] for c in range(N_CORES)], axis=0
    ).astype(np.float64)                                   # [64, 512]
    den_neg = np.concatenate(
        [res.results[c]["den_out"][:PPC, 0] for c in range(N_CORES)]
    ).astype(np.float64)                                   # [56]

    s_i, s_j = sums[:P], sums[L:]
    na = np.maximum(np.sqrt((s_i * s_i).sum(1)), EPS)
    nb = np.maximum(np.sqrt((s_j * s_j).sum(1)), EPS)
    pos = (s_i * s_j).sum(1) / (na * nb)
    num = np.exp(pos / TEMP)
    den = num + den_neg
    total = 2.0 * np.sum(np.log(den) - pos / TEMP)
    return np.asarray(total, dtype=np.float32)

